# revision 4
# baseline (speedup 1.0000x reference)
"""CQAttention (QANet context-query attention) Trainium2 kernel.

Problem: B=64, H=256, Lc=2048, Lq=256.
  S[b,i,j] = (Ct@w1)[i] + (Qt@w2)[j] + sum_h Ct[i,h]*w3[h]*Qt[j,h]
  S_row = softmax_j(masked), S_col = softmax_i(masked)
  A = S_row @ Qt ; Bt = S_row @ (S_col^T @ Ct)
  out[b] = [Ct; A; Ct*A; Ct*Bt]^T  -> [B, 4H, Lc]

Strategy: data-parallel over batch (8 per core x 8 cores). Per batch:
  - host precomputes r=Ct@w1, c=Qt@w2, mask-folded bias columns, Qt, Q*w3,
    and bf16 Ct augmented with a ones column (for column-softmax sums).
  - S^T [j,i] on PE (lhsT=Q*w3, rhs=C) -> ACT exp with per-partition bias
    (c[j] - 1e30*qmask[j]) -> Pr^T (float32r, unnormalized).
  - row sums replicated across partitions via ones-matmul; reciprocal on DVE.
  - S [i,j] on PE (lhsT=C, rhs=Q*w3) -> ACT exp with bias
    (r[i] - 1e30*cmask[i]) -> Pc (bf16).
  - X_aug = Pc^T @ [Ct|1] (bf16) gives col-attention numerator + colsum;
    normalized on eviction (tensor_scalar by 1/colsum).
  - A^T = Qt^T @ Pr^T and Bt^T = X^T @ Pr^T (f32r), row-normalized by the
    replicated reciprocal during PSUM eviction (DVE tensor_tensor).
  - epilogue products with C split across GPSIMD/DVE; 1MB output DMAs.
"""

import numpy as np

B, H, LC, LQ = 64, 256, 2048, 256
NCORES = 8
NB = B // NCORES  # batches per core
NEG = 1.0e30

HC = H // 128   # 2 h-chunks
JC = LQ // 128  # 2 j-chunks
IC = LC // 128  # 16 i-chunks
IT = LC // 512  # 4 i-tiles
HA = H + 1      # augmented (ones column) width

_CACHE = {}


def _build():
    import concourse.bacc as bacc
    import concourse.mybir as mybir
    import concourse.tile as tile
    from contextlib import ExitStack

    F32 = mybir.dt.float32
    F32R = mybir.dt.float32r
    BF16 = mybir.dt.bfloat16
    AF = mybir.ActivationFunctionType
    MUL = mybir.AluOpType.mult

    nc = bacc.Bacc("TRN2", target_bir_lowering=False, debug=False,
                   enable_asserts=False)

    c32 = nc.dram_tensor("c32", [NB, H, LC], F32, kind="ExternalInput").ap()
    ct = nc.dram_tensor("ct", [NB, LC, HA], BF16, kind="ExternalInput").ap()
    q3 = nc.dram_tensor("q3", [NB, H, LQ], F32, kind="ExternalInput").ap()
    qt = nc.dram_tensor("qt", [NB, LQ, H], F32, kind="ExternalInput").ap()
    rm = nc.dram_tensor("rm", [NB, 128, IC], F32, kind="ExternalInput").ap()
    cb = nc.dram_tensor("cb", [NB, 128, JC], F32, kind="ExternalInput").ap()
    out = nc.dram_tensor("out", [NB, 4 * H, LC], F32, kind="ExternalOutput").ap()

    with tile.TileContext(nc) as tc:
        with ExitStack() as ctx:
            konst = ctx.enter_context(tc.tile_pool(name="konst", bufs=1))
            crpool = ctx.enter_context(tc.tile_pool(name="crpool", bufs=2))
            ctpool = ctx.enter_context(tc.tile_pool(name="ctpool", bufs=2))
            qpool = ctx.enter_context(tc.tile_pool(name="qpool", bufs=2))
            prpool = ctx.enter_context(tc.tile_pool(name="prpool", bufs=2))
            pcpool = ctx.enter_context(tc.tile_pool(name="pcpool", bufs=2))
            rrpool = ctx.enter_context(tc.tile_pool(name="rrpool", bufs=2))
            xpool = ctx.enter_context(tc.tile_pool(name="xpool", bufs=2))
            opool = ctx.enter_context(tc.tile_pool(name="opool", bufs=6))
            small = ctx.enter_context(tc.tile_pool(name="small", bufs=4))
            mm_ps = ctx.enter_context(tc.tile_pool(name="mm_ps", bufs=4, space="PSUM"))
            s3_ps = ctx.enter_context(tc.tile_pool(name="s3_ps", bufs=2, space="PSUM"))
            x_ps = ctx.enter_context(tc.tile_pool(name="x_ps", bufs=2, space="PSUM"))

            ones32 = konst.tile([128, 128], F32)
            nc.vector.memset(ones32[:], 1.0)
            ones_r = konst.tile([128, 128], F32R)
            nc.vector.tensor_copy(ones_r[:], ones32[:])

            for b in range(NB):
                # ---- loads ----
                crsb = crpool.tile([128, HC * LC], F32R, tag="crsb")
                nc.gpsimd.dma_start(
                    crsb[:].rearrange("p (c i) -> p c i", c=HC),
                    c32[b].rearrange("(c p) i -> p c i", p=128))
                cf = crsb[:].bitcast(F32)  # rounded C, fp32 view for epilogue
                ctsb = ctpool.tile([128, IC * HA], BF16, tag="ctsb")
                nc.sync.dma_start(
                    ctsb[:].rearrange("p (n h) -> p n h", n=IC),
                    ct[b].rearrange("(n p) h -> p n h", p=128))
                q3sb = qpool.tile([128, HC * LQ], F32R, tag="q3sb")
                nc.gpsimd.dma_start(
                    q3sb[:].rearrange("p (c j) -> p c j", c=HC),
                    q3[b].rearrange("(c p) j -> p c j", p=128))
                qtsb = qpool.tile([128, JC * H], F32R, tag="qtsb")
                nc.gpsimd.dma_start(
                    qtsb[:].rearrange("p (c h) -> p c h", c=JC),
                    qt[b].rearrange("(c p) h -> p c h", p=128))
                rmsb = small.tile([128, IC], F32, tag="rmsb")
                nc.sync.dma_start(rmsb[:], rm[b])
                cbsb = small.tile([128, JC], F32, tag="cbsb")
                nc.sync.dma_start(cbsb[:], cb[b])

                # ---- row path: S^T tiles -> exp -> Pr^T; replicated rowsums ----
                prt = prpool.tile([128, JC * LC], F32R, tag="prt")
                rrep = rrpool.tile([128, LC], F32, tag="rrep")
                for it in range(IT):
                    for jc in range(JC):
                        ps = mm_ps.tile([128, 512], F32, tag="mm")
                        for kc in range(HC):
                            nc.tensor.matmul(
                                ps[:],
                                q3sb[:, kc * LQ + jc * 128:kc * LQ + (jc + 1) * 128],
                                crsb[:, kc * LC + it * 512:kc * LC + (it + 1) * 512],
                                start=(kc == 0), stop=(kc == HC - 1))
                        nc.scalar.activation(
                            prt[:, jc * LC + it * 512:jc * LC + (it + 1) * 512],
                            ps[:], AF.Exp, bias=cbsb[:, jc:jc + 1])
                    rs = mm_ps.tile([128, 512], F32, tag="mm")
                    for jc in range(JC):
                        nc.tensor.matmul(
                            rs[:], ones_r[:],
                            prt[:, jc * LC + it * 512:jc * LC + (it + 1) * 512],
                            start=(jc == 0), stop=(jc == JC - 1))
                    nc.vector.reciprocal(rrep[:, it * 512:(it + 1) * 512], rs[:])

                # ---- col path: S tiles -> exp -> Pc (bf16) ----
                pc = pcpool.tile([128, IC * LQ], BF16, tag="pc")
                for ic in range(IC):
                    ps3 = s3_ps.tile([128, LQ], F32, tag="s3")
                    for kc in range(HC):
                        nc.tensor.matmul(
                            ps3[:],
                            crsb[:, kc * LC + ic * 128:kc * LC + (ic + 1) * 128],
                            q3sb[:, kc * LQ:(kc + 1) * LQ],
                            start=(kc == 0), stop=(kc == HC - 1))
                    nc.scalar.activation(
                        pc[:, ic * LQ:(ic + 1) * LQ],
                        ps3[:], AF.Exp, bias=rmsb[:, ic:ic + 1])

                # ---- M3: X_aug = Pc^T @ [Ct|1]; normalize by colsum ----
                xsb = xpool.tile([128, JC * H], F32R, tag="xsb")
                for jc in range(JC):
                    xps = x_ps.tile([128, HA], F32, tag="x")
                    for ic in range(IC):
                        nc.tensor.matmul(
                            xps[:],
                            pc[:, ic * LQ + jc * 128:ic * LQ + (jc + 1) * 128],
                            ctsb[:, ic * HA:(ic + 1) * HA],
                            start=(ic == 0), stop=(ic == IC - 1))
                    colr = small.tile([128, 1], F32, tag="colr")
                    nc.vector.reciprocal(colr[:], xps[:, H:H + 1])
                    nc.vector.tensor_scalar_mul(
                        xsb[:, jc * H:(jc + 1) * H], xps[:, 0:H], colr[:])

                # ---- M2/M4 + epilogue ----
                for hc in range(HC):
                    o2 = opool.tile([128, LC], F32, tag="obuf")
                    o3 = opool.tile([128, LC], F32, tag="obuf")
                    o4 = opool.tile([128, LC], F32, tag="obuf")
                    for it in range(IT):
                        i0, i1 = it * 512, (it + 1) * 512
                        aps = mm_ps.tile([128, 512], F32, tag="mm")
                        for jc in range(JC):
                            nc.tensor.matmul(
                                aps[:],
                                qtsb[:, jc * H + hc * 128:jc * H + (hc + 1) * 128],
                                prt[:, jc * LC + i0:jc * LC + i1],
                                start=(jc == 0), stop=(jc == JC - 1))
                        bps = mm_ps.tile([128, 512], F32, tag="mm")
                        for jc in range(JC):
                            nc.tensor.matmul(
                                bps[:],
                                xsb[:, jc * H + hc * 128:jc * H + (hc + 1) * 128],
                                prt[:, jc * LC + i0:jc * LC + i1],
                                start=(jc == 0), stop=(jc == JC - 1))
                        # O2 = A^T*rrep ; O4 = Bt^T*(C*rrep) ; O3 = O2*C
                        nc.vector.tensor_tensor(
                            o2[:, i0:i1], aps[:], rrep[:, i0:i1], MUL)
                        cr = small.tile([128, 512], F32, tag="cr")
                        nc.gpsimd.tensor_tensor(
                            cr[:], cf[:, hc * LC + i0:hc * LC + i1],
                            rrep[:, i0:i1], MUL)
                        nc.vector.tensor_tensor(o4[:, i0:i1], bps[:], cr[:], MUL)
                        nc.gpsimd.tensor_tensor(
                            o3[:, i0:i1], o2[:, i0:i1],
                            cf[:, hc * LC + i0:hc * LC + i1], MUL)
                    # ---- stores ----
                    r0 = hc * 128
                    nc.scalar.dma_start(out[b, r0:r0 + 128, :],
                                        cf[:, hc * LC:(hc + 1) * LC])
                    nc.scalar.dma_start(out[b, H + r0:H + r0 + 128, :], o2[:])
                    nc.scalar.dma_start(out[b, 2 * H + r0:2 * H + r0 + 128, :], o3[:])
                    nc.scalar.dma_start(out[b, 3 * H + r0:3 * H + r0 + 128, :], o4[:])

    nc.compile()
    return nc


def _prep(C, Q, cmask, qmask, line_project):
    import ml_dtypes
    w1, w2, w3 = np.split(line_project.astype(np.float64), 3)
    r = np.einsum('bhi,h->bi', C.astype(np.float64), w1).astype(np.float32)
    c_ = np.einsum('bhj,h->bj', Q.astype(np.float64), w2).astype(np.float32)
    rm = (r - NEG * cmask).reshape(B, IC, 128).transpose(0, 2, 1).copy()
    cb = (c_ - NEG * qmask).reshape(B, JC, 128).transpose(0, 2, 1).copy()
    q3 = (Q * w3.astype(np.float32)[None, :, None]).astype(np.float32)
    qt = np.ascontiguousarray(Q.transpose(0, 2, 1))
    ct = np.empty((B, LC, HA), dtype=ml_dtypes.bfloat16)
    ct[:, :, :H] = C.transpose(0, 2, 1).astype(ml_dtypes.bfloat16)
    ct[:, :, H] = np.float32(1.0)
    return rm, cb, q3, qt, ct


def kernel(C, Q, cmask, qmask, line_project):
    from concourse.bass_utils import run_bass_kernel_spmd

    C = np.asarray(C, dtype=np.float32)
    Q = np.asarray(Q, dtype=np.float32)
    cmask = np.asarray(cmask, dtype=np.float32)
    qmask = np.asarray(qmask, dtype=np.float32)
    line_project = np.asarray(line_project, dtype=np.float32)

    rm, cb, q3, qt, ct = _prep(C, Q, cmask, qmask, line_project)

    if "nc" not in _CACHE:
        _CACHE["nc"] = _build()
    nc = _CACHE["nc"]

    in_maps = []
    for core in range(NCORES):
        s = slice(core * NB, (core + 1) * NB)
        in_maps.append({
            "c32": np.ascontiguousarray(C[s]),
            "ct": np.ascontiguousarray(ct[s]),
            "q3": np.ascontiguousarray(q3[s]),
            "qt": np.ascontiguousarray(qt[s]),
            "rm": np.ascontiguousarray(rm[s]),
            "cb": np.ascontiguousarray(cb[s]),
        })
    res = run_bass_kernel_spmd(nc, in_maps, core_ids=list(range(NCORES)))
    return np.concatenate([res.results[c]["out"] for c in range(NCORES)], axis=0)


# revision 6
# speedup vs baseline: 1.1000x; 1.1000x over previous
"""CQAttention (QANet context-query attention) Trainium2 kernel.

Problem: B=64, H=256, Lc=2048, Lq=256.
  S[b,i,j] = (Ct@w1)[i] + (Qt@w2)[j] + sum_h Ct[i,h]*w3[h]*Qt[j,h]
  S_row = softmax_j(masked), S_col = softmax_i(masked)
  A = S_row @ Qt ; Bt = S_row @ (S_col^T @ Ct)
  out[b] = [Ct; A; Ct*A; Ct*Bt]^T  -> [B, 4H, Lc]

Strategy: data-parallel over batch (8 per core x 8 cores). Per batch:
  - host precomputes r=Ct@w1, c=Qt@w2, mask-folded bias columns, Qt, Q*w3,
    and bf16 Ct augmented with a ones column (for column-softmax sums).
  - S^T [j,i] on PE (lhsT=Q*w3, rhs=C) -> ACT exp with per-partition bias
    (c[j] - 1e30*qmask[j]) -> Pr^T (float32r, unnormalized).
  - row sums replicated across partitions via ones-matmul; reciprocal on DVE.
  - S [i,j] on PE (lhsT=C, rhs=Q*w3) -> ACT exp with bias
    (r[i] - 1e30*cmask[i]) -> Pc (bf16).
  - X_aug = Pc^T @ [Ct|1] (bf16) gives col-attention numerator + colsum;
    normalized on eviction (tensor_scalar by 1/colsum).
  - A^T = Qt^T @ Pr^T and Bt^T = X^T @ Pr^T (f32r), row-normalized by the
    replicated reciprocal during PSUM eviction (DVE tensor_tensor).
  - epilogue products with C split across GPSIMD/DVE; 1MB output DMAs.
"""

import numpy as np

B, H, LC, LQ = 64, 256, 2048, 256
NCORES = 8
NB = B // NCORES  # batches per core
NEG = 1.0e30

HC = H // 128   # 2 h-chunks
JC = LQ // 128  # 2 j-chunks
IC = LC // 128  # 16 i-chunks
IT = LC // 512  # 4 i-tiles
HA = H + 1      # augmented (ones column) width

_CACHE = {}


def _build():
    import concourse.bacc as bacc
    import concourse.mybir as mybir
    import concourse.tile as tile
    from contextlib import ExitStack

    F32 = mybir.dt.float32
    F32R = mybir.dt.float32r
    BF16 = mybir.dt.bfloat16
    AF = mybir.ActivationFunctionType
    MUL = mybir.AluOpType.mult

    nc = bacc.Bacc("TRN2", target_bir_lowering=False, debug=False,
                   enable_asserts=False)

    c32 = nc.dram_tensor("c32", [NB, H, LC], F32, kind="ExternalInput").ap()
    ct = nc.dram_tensor("ct", [NB, LC, HA], BF16, kind="ExternalInput").ap()
    q3 = nc.dram_tensor("q3", [NB, H, LQ], F32, kind="ExternalInput").ap()
    qt = nc.dram_tensor("qt", [NB, LQ, H], F32, kind="ExternalInput").ap()
    rm = nc.dram_tensor("rm", [NB, 128, IC], F32, kind="ExternalInput").ap()
    cb = nc.dram_tensor("cb", [NB, 128, JC], F32, kind="ExternalInput").ap()
    out = nc.dram_tensor("out", [NB, 4 * H, LC], F32, kind="ExternalOutput").ap()

    with tile.TileContext(nc) as tc:
        with ExitStack() as ctx:
            konst = ctx.enter_context(tc.tile_pool(name="konst", bufs=1))
            crpool = ctx.enter_context(tc.tile_pool(name="crpool", bufs=2))
            ctpool = ctx.enter_context(tc.tile_pool(name="ctpool", bufs=2))
            qpool = ctx.enter_context(tc.tile_pool(name="qpool", bufs=2))
            prpool = ctx.enter_context(tc.tile_pool(name="prpool", bufs=2))
            pcpool = ctx.enter_context(tc.tile_pool(name="pcpool", bufs=2))
            rrpool = ctx.enter_context(tc.tile_pool(name="rrpool", bufs=2))
            xpool = ctx.enter_context(tc.tile_pool(name="xpool", bufs=2))
            opool = ctx.enter_context(tc.tile_pool(name="opool", bufs=6))
            small = ctx.enter_context(tc.tile_pool(name="small", bufs=4))
            mm_ps = ctx.enter_context(tc.tile_pool(name="mm_ps", bufs=4, space="PSUM"))
            s3_ps = ctx.enter_context(tc.tile_pool(name="s3_ps", bufs=2, space="PSUM"))
            x_ps = ctx.enter_context(tc.tile_pool(name="x_ps", bufs=2, space="PSUM"))

            ones32 = konst.tile([128, 128], F32)
            nc.vector.memset(ones32[:], 1.0)
            ones_r = konst.tile([128, 128], F32R)
            nc.vector.tensor_copy(ones_r[:], ones32[:])

            def load_batch(b):
                crsb = crpool.tile([128, HC * LC], F32R, tag="crsb")
                nc.gpsimd.dma_start(
                    crsb[:].rearrange("p (c i) -> p c i", c=HC),
                    c32[b].rearrange("(c p) i -> p c i", p=128))
                ctsb = ctpool.tile([128, IC * HA], BF16, tag="ctsb")
                nc.sync.dma_start(
                    ctsb[:].rearrange("p (n h) -> p n h", n=IC),
                    ct[b].rearrange("(n p) h -> p n h", p=128))
                q3sb = qpool.tile([128, HC * LQ], F32R, tag="q3sb")
                nc.gpsimd.dma_start(
                    q3sb[:].rearrange("p (c j) -> p c j", c=HC),
                    q3[b].rearrange("(c p) j -> p c j", p=128))
                qtsb = qpool.tile([128, JC * H], F32R, tag="qtsb")
                nc.gpsimd.dma_start(
                    qtsb[:].rearrange("p (c h) -> p c h", c=JC),
                    qt[b].rearrange("(c p) h -> p c h", p=128))
                rmsb = small.tile([128, IC], F32, tag="rmsb")
                nc.sync.dma_start(rmsb[:], rm[b])
                cbsb = small.tile([128, JC], F32, tag="cbsb")
                nc.sync.dma_start(cbsb[:], cb[b])
                return crsb, ctsb, q3sb, qtsb, rmsb, cbsb

            tiles = load_batch(0)
            for b in range(NB):
                crsb, ctsb, q3sb, qtsb, rmsb, cbsb = tiles
                cf = crsb[:].bitcast(F32)  # rounded C, fp32 view for epilogue
                if b + 1 < NB:
                    tiles = load_batch(b + 1)

                # ---- row path: S^T tiles -> exp -> Pr^T; replicated rowsums ----
                prt = prpool.tile([128, JC * LC], F32R, tag="prt")
                rrep = rrpool.tile([128, LC], F32, tag="rrep")
                for it in range(IT):
                    for jc in range(JC):
                        ps = mm_ps.tile([128, 512], F32, tag="mm")
                        for kc in range(HC):
                            nc.tensor.matmul(
                                ps[:],
                                q3sb[:, kc * LQ + jc * 128:kc * LQ + (jc + 1) * 128],
                                crsb[:, kc * LC + it * 512:kc * LC + (it + 1) * 512],
                                start=(kc == 0), stop=(kc == HC - 1))
                        nc.scalar.activation(
                            prt[:, jc * LC + it * 512:jc * LC + (it + 1) * 512],
                            ps[:], AF.Exp, bias=cbsb[:, jc:jc + 1])
                    rs = mm_ps.tile([128, 512], F32, tag="mm")
                    for jc in range(JC):
                        nc.tensor.matmul(
                            rs[:], ones_r[:],
                            prt[:, jc * LC + it * 512:jc * LC + (it + 1) * 512],
                            start=(jc == 0), stop=(jc == JC - 1))
                    nc.vector.reciprocal(rrep[:, it * 512:(it + 1) * 512], rs[:])

                # ---- col path: S tiles -> exp -> Pc (bf16) ----
                pc = pcpool.tile([128, IC * LQ], BF16, tag="pc")
                for ic in range(IC):
                    ps3 = s3_ps.tile([128, LQ], F32, tag="s3")
                    for kc in range(HC):
                        nc.tensor.matmul(
                            ps3[:],
                            crsb[:, kc * LC + ic * 128:kc * LC + (ic + 1) * 128],
                            q3sb[:, kc * LQ:(kc + 1) * LQ],
                            start=(kc == 0), stop=(kc == HC - 1))
                    nc.scalar.activation(
                        pc[:, ic * LQ:(ic + 1) * LQ],
                        ps3[:], AF.Exp, bias=rmsb[:, ic:ic + 1])

                # ---- M3: X_aug = Pc^T @ [Ct|1]; normalize by colsum ----
                xsb = xpool.tile([128, JC * H], F32R, tag="xsb")
                for jc in range(JC):
                    xps = x_ps.tile([128, HA], F32, tag="x")
                    for ic in range(IC):
                        nc.tensor.matmul(
                            xps[:],
                            pc[:, ic * LQ + jc * 128:ic * LQ + (jc + 1) * 128],
                            ctsb[:, ic * HA:(ic + 1) * HA],
                            start=(ic == 0), stop=(ic == IC - 1))
                    colr = small.tile([128, 1], F32, tag="colr")
                    nc.vector.reciprocal(colr[:], xps[:, H:H + 1])
                    nc.vector.tensor_scalar_mul(
                        xsb[:, jc * H:(jc + 1) * H], xps[:, 0:H], colr[:])

                # ---- M2/M4 + epilogue ----
                for hc in range(HC):
                    o2 = opool.tile([128, LC], F32, tag="obuf")
                    o3 = opool.tile([128, LC], F32, tag="obuf")
                    o4 = opool.tile([128, LC], F32, tag="obuf")
                    for it in range(IT):
                        i0, i1 = it * 512, (it + 1) * 512
                        aps = mm_ps.tile([128, 512], F32, tag="mm")
                        for jc in range(JC):
                            nc.tensor.matmul(
                                aps[:],
                                qtsb[:, jc * H + hc * 128:jc * H + (hc + 1) * 128],
                                prt[:, jc * LC + i0:jc * LC + i1],
                                start=(jc == 0), stop=(jc == JC - 1))
                        bps = mm_ps.tile([128, 512], F32, tag="mm")
                        for jc in range(JC):
                            nc.tensor.matmul(
                                bps[:],
                                xsb[:, jc * H + hc * 128:jc * H + (hc + 1) * 128],
                                prt[:, jc * LC + i0:jc * LC + i1],
                                start=(jc == 0), stop=(jc == JC - 1))
                        # O2 = A^T*rrep ; O4 = Bt^T*(C*rrep) ; O3 = O2*C
                        nc.vector.tensor_tensor(
                            o2[:, i0:i1], aps[:], rrep[:, i0:i1], MUL)
                        cr = small.tile([128, 512], F32, tag="cr")
                        nc.gpsimd.tensor_tensor(
                            cr[:], cf[:, hc * LC + i0:hc * LC + i1],
                            rrep[:, i0:i1], MUL)
                        nc.vector.tensor_tensor(o4[:, i0:i1], bps[:], cr[:], MUL)
                        nc.gpsimd.tensor_tensor(
                            o3[:, i0:i1], o2[:, i0:i1],
                            cf[:, hc * LC + i0:hc * LC + i1], MUL)
                    # ---- stores ----
                    r0 = hc * 128
                    nc.sync.dma_start(out[b, r0:r0 + 128, :],
                                      cf[:, hc * LC:(hc + 1) * LC])
                    nc.sync.dma_start(out[b, H + r0:H + r0 + 128, :], o2[:])
                    nc.sync.dma_start(out[b, 2 * H + r0:2 * H + r0 + 128, :], o3[:])
                    nc.sync.dma_start(out[b, 3 * H + r0:3 * H + r0 + 128, :], o4[:])

    nc.compile()
    return nc


def _prep(C, Q, cmask, qmask, line_project):
    import ml_dtypes
    w1, w2, w3 = np.split(line_project.astype(np.float64), 3)
    r = np.einsum('bhi,h->bi', C.astype(np.float64), w1).astype(np.float32)
    c_ = np.einsum('bhj,h->bj', Q.astype(np.float64), w2).astype(np.float32)
    rm = (r - NEG * cmask).reshape(B, IC, 128).transpose(0, 2, 1).copy()
    cb = (c_ - NEG * qmask).reshape(B, JC, 128).transpose(0, 2, 1).copy()
    q3 = (Q * w3.astype(np.float32)[None, :, None]).astype(np.float32)
    qt = np.ascontiguousarray(Q.transpose(0, 2, 1))
    ct = np.empty((B, LC, HA), dtype=ml_dtypes.bfloat16)
    ct[:, :, :H] = C.transpose(0, 2, 1).astype(ml_dtypes.bfloat16)
    ct[:, :, H] = np.float32(1.0)
    return rm, cb, q3, qt, ct


def kernel(C, Q, cmask, qmask, line_project):
    from concourse.bass_utils import run_bass_kernel_spmd

    C = np.asarray(C, dtype=np.float32)
    Q = np.asarray(Q, dtype=np.float32)
    cmask = np.asarray(cmask, dtype=np.float32)
    qmask = np.asarray(qmask, dtype=np.float32)
    line_project = np.asarray(line_project, dtype=np.float32)

    rm, cb, q3, qt, ct = _prep(C, Q, cmask, qmask, line_project)

    if "nc" not in _CACHE:
        _CACHE["nc"] = _build()
    nc = _CACHE["nc"]

    in_maps = []
    for core in range(NCORES):
        s = slice(core * NB, (core + 1) * NB)
        in_maps.append({
            "c32": np.ascontiguousarray(C[s]),
            "ct": np.ascontiguousarray(ct[s]),
            "q3": np.ascontiguousarray(q3[s]),
            "qt": np.ascontiguousarray(qt[s]),
            "rm": np.ascontiguousarray(rm[s]),
            "cb": np.ascontiguousarray(cb[s]),
        })
    res = run_bass_kernel_spmd(nc, in_maps, core_ids=list(range(NCORES)))
    return np.concatenate([res.results[c]["out"] for c in range(NCORES)], axis=0)


# revision 7
# speedup vs baseline: 1.1936x; 1.0851x over previous
"""CQAttention (QANet context-query attention) Trainium2 kernel.

Problem: B=64, H=256, Lc=2048, Lq=256.
  S[b,i,j] = (Ct@w1)[i] + (Qt@w2)[j] + sum_h Ct[i,h]*w3[h]*Qt[j,h]
  S_row = softmax_j(masked), S_col = softmax_i(masked)
  A = S_row @ Qt ; Bt = S_row @ (S_col^T @ Ct)
  out[b] = [Ct; A; Ct*A; Ct*Bt]^T  -> [B, 4H, Lc]

Strategy: data-parallel over batch (8 per core x 8 cores). Per batch:
  - host precomputes r=Ct@w1, c=Qt@w2, mask-folded bias columns, Qt, Q*w3,
    and bf16 Ct augmented with a ones column (for column-softmax sums).
  - S^T [j,i] on PE (lhsT=Q*w3, rhs=C) -> ACT exp with per-partition bias
    (c[j] - 1e30*qmask[j]) -> Pr^T (float32r, unnormalized).
  - row sums replicated across partitions via ones-matmul; reciprocal on DVE.
  - S [i,j] on PE (lhsT=C, rhs=Q*w3) -> ACT exp with bias
    (r[i] - 1e30*cmask[i]) -> Pc (bf16).
  - X_aug = Pc^T @ [Ct|1] (bf16) gives col-attention numerator + colsum;
    normalized on eviction (tensor_scalar by 1/colsum).
  - A^T = Qt^T @ Pr^T and Bt^T = X^T @ Pr^T (f32r), row-normalized by the
    replicated reciprocal during PSUM eviction (DVE tensor_tensor).
  - epilogue products with C split across GPSIMD/DVE; 1MB output DMAs.
"""

import numpy as np

B, H, LC, LQ = 64, 256, 2048, 256
NCORES = 8
NB = B // NCORES  # batches per core
NEG = 1.0e30

HC = H // 128   # 2 h-chunks
JC = LQ // 128  # 2 j-chunks
IC = LC // 128  # 16 i-chunks
IT = LC // 512  # 4 i-tiles
HA = H + 1      # augmented (ones column) width

_CACHE = {}


def _build():
    import concourse.bacc as bacc
    import concourse.mybir as mybir
    import concourse.tile as tile
    from contextlib import ExitStack

    F32 = mybir.dt.float32
    F32R = mybir.dt.float32r
    BF16 = mybir.dt.bfloat16
    AF = mybir.ActivationFunctionType
    MUL = mybir.AluOpType.mult

    nc = bacc.Bacc("TRN2", target_bir_lowering=False, debug=False,
                   enable_asserts=False)

    c32 = nc.dram_tensor("c32", [NB, H, LC], F32R, kind="ExternalInput").ap()
    ct = nc.dram_tensor("ct", [NB, LC, HA], BF16, kind="ExternalInput").ap()
    q3 = nc.dram_tensor("q3", [NB, H, LQ], F32R, kind="ExternalInput").ap()
    qt = nc.dram_tensor("qt", [NB, LQ, H], F32R, kind="ExternalInput").ap()
    rm = nc.dram_tensor("rm", [NB, 128, IC], F32, kind="ExternalInput").ap()
    cb = nc.dram_tensor("cb", [NB, 128, JC], F32, kind="ExternalInput").ap()
    out = nc.dram_tensor("out", [NB, 4 * H, LC], F32, kind="ExternalOutput").ap()

    with tile.TileContext(nc) as tc:
        with ExitStack() as ctx:
            konst = ctx.enter_context(tc.tile_pool(name="konst", bufs=1))
            crpool = ctx.enter_context(tc.tile_pool(name="crpool", bufs=2))
            ctpool = ctx.enter_context(tc.tile_pool(name="ctpool", bufs=2))
            qpool = ctx.enter_context(tc.tile_pool(name="qpool", bufs=2))
            prpool = ctx.enter_context(tc.tile_pool(name="prpool", bufs=2))
            pcpool = ctx.enter_context(tc.tile_pool(name="pcpool", bufs=2))
            rrpool = ctx.enter_context(tc.tile_pool(name="rrpool", bufs=2))
            xpool = ctx.enter_context(tc.tile_pool(name="xpool", bufs=2))
            opool = ctx.enter_context(tc.tile_pool(name="opool", bufs=6))
            small = ctx.enter_context(tc.tile_pool(name="small", bufs=4))
            mm_ps = ctx.enter_context(tc.tile_pool(name="mm_ps", bufs=4, space="PSUM"))
            s3_ps = ctx.enter_context(tc.tile_pool(name="s3_ps", bufs=2, space="PSUM"))
            x_ps = ctx.enter_context(tc.tile_pool(name="x_ps", bufs=2, space="PSUM"))

            ones32 = konst.tile([128, 128], F32)
            nc.vector.memset(ones32[:], 1.0)
            ones_r = konst.tile([128, 128], F32R)
            nc.vector.tensor_copy(ones_r[:], ones32[:])

            def load_batch(b):
                crsb = crpool.tile([128, HC * LC], F32R, tag="crsb")
                nc.sync.dma_start(
                    crsb[:].rearrange("p (c i) -> p c i", c=HC),
                    c32[b].rearrange("(c p) i -> p c i", p=128))
                ctsb = ctpool.tile([128, IC * HA], BF16, tag="ctsb")
                nc.sync.dma_start(
                    ctsb[:].rearrange("p (n h) -> p n h", n=IC),
                    ct[b].rearrange("(n p) h -> p n h", p=128))
                q3sb = qpool.tile([128, HC * LQ], F32R, tag="q3sb")
                nc.sync.dma_start(
                    q3sb[:].rearrange("p (c j) -> p c j", c=HC),
                    q3[b].rearrange("(c p) j -> p c j", p=128))
                qtsb = qpool.tile([128, JC * H], F32R, tag="qtsb")
                nc.sync.dma_start(
                    qtsb[:].rearrange("p (c h) -> p c h", c=JC),
                    qt[b].rearrange("(c p) h -> p c h", p=128))
                rmsb = small.tile([128, IC], F32, tag="rmsb")
                nc.sync.dma_start(rmsb[:], rm[b])
                cbsb = small.tile([128, JC], F32, tag="cbsb")
                nc.sync.dma_start(cbsb[:], cb[b])
                return crsb, ctsb, q3sb, qtsb, rmsb, cbsb

            tiles = load_batch(0)
            for b in range(NB):
                crsb, ctsb, q3sb, qtsb, rmsb, cbsb = tiles
                cf = crsb[:].bitcast(F32)  # rounded C, fp32 view for epilogue
                if b + 1 < NB:
                    tiles = load_batch(b + 1)

                # ---- row path: S^T tiles -> exp -> Pr^T; replicated rowsums ----
                prt = prpool.tile([128, JC * LC], F32R, tag="prt")
                rrep = rrpool.tile([128, LC], F32, tag="rrep")
                for it in range(IT):
                    for jc in range(JC):
                        ps = mm_ps.tile([128, 512], F32, tag="mm")
                        for kc in range(HC):
                            nc.tensor.matmul(
                                ps[:],
                                q3sb[:, kc * LQ + jc * 128:kc * LQ + (jc + 1) * 128],
                                crsb[:, kc * LC + it * 512:kc * LC + (it + 1) * 512],
                                start=(kc == 0), stop=(kc == HC - 1))
                        nc.scalar.activation(
                            prt[:, jc * LC + it * 512:jc * LC + (it + 1) * 512],
                            ps[:], AF.Exp, bias=cbsb[:, jc:jc + 1])
                    rs = mm_ps.tile([128, 512], F32, tag="mm")
                    for jc in range(JC):
                        nc.tensor.matmul(
                            rs[:], ones_r[:],
                            prt[:, jc * LC + it * 512:jc * LC + (it + 1) * 512],
                            start=(jc == 0), stop=(jc == JC - 1))
                    nc.vector.reciprocal(rrep[:, it * 512:(it + 1) * 512], rs[:])

                # ---- col path: S tiles -> exp -> Pc (bf16) ----
                pc = pcpool.tile([128, IC * LQ], BF16, tag="pc")
                for ic in range(IC):
                    ps3 = s3_ps.tile([128, LQ], F32, tag="s3")
                    for kc in range(HC):
                        nc.tensor.matmul(
                            ps3[:],
                            crsb[:, kc * LC + ic * 128:kc * LC + (ic + 1) * 128],
                            q3sb[:, kc * LQ:(kc + 1) * LQ],
                            start=(kc == 0), stop=(kc == HC - 1))
                    nc.scalar.activation(
                        pc[:, ic * LQ:(ic + 1) * LQ],
                        ps3[:], AF.Exp, bias=rmsb[:, ic:ic + 1])

                # ---- M3: X_aug = Pc^T @ [Ct|1]; normalize by colsum ----
                xsb = xpool.tile([128, JC * H], F32R, tag="xsb")
                for jc in range(JC):
                    xps = x_ps.tile([128, HA], F32, tag="x")
                    for ic in range(IC):
                        nc.tensor.matmul(
                            xps[:],
                            pc[:, ic * LQ + jc * 128:ic * LQ + (jc + 1) * 128],
                            ctsb[:, ic * HA:(ic + 1) * HA],
                            start=(ic == 0), stop=(ic == IC - 1))
                    colr = small.tile([128, 1], F32, tag="colr")
                    nc.vector.reciprocal(colr[:], xps[:, H:H + 1])
                    nc.vector.tensor_scalar_mul(
                        xsb[:, jc * H:(jc + 1) * H], xps[:, 0:H], colr[:])

                # ---- M2/M4 + epilogue ----
                for hc in range(HC):
                    o2 = opool.tile([128, LC], F32, tag="obuf")
                    o3 = opool.tile([128, LC], F32, tag="obuf")
                    o4 = opool.tile([128, LC], F32, tag="obuf")
                    for it in range(IT):
                        i0, i1 = it * 512, (it + 1) * 512
                        aps = mm_ps.tile([128, 512], F32, tag="mm")
                        for jc in range(JC):
                            nc.tensor.matmul(
                                aps[:],
                                qtsb[:, jc * H + hc * 128:jc * H + (hc + 1) * 128],
                                prt[:, jc * LC + i0:jc * LC + i1],
                                start=(jc == 0), stop=(jc == JC - 1))
                        bps = mm_ps.tile([128, 512], F32, tag="mm")
                        for jc in range(JC):
                            nc.tensor.matmul(
                                bps[:],
                                xsb[:, jc * H + hc * 128:jc * H + (hc + 1) * 128],
                                prt[:, jc * LC + i0:jc * LC + i1],
                                start=(jc == 0), stop=(jc == JC - 1))
                        # O2 = A^T*rrep ; O4 = Bt^T*(C*rrep) ; O3 = O2*C
                        nc.vector.tensor_tensor(
                            o2[:, i0:i1], aps[:], rrep[:, i0:i1], MUL)
                        cr = small.tile([128, 512], F32, tag="cr")
                        nc.gpsimd.tensor_tensor(
                            cr[:], cf[:, hc * LC + i0:hc * LC + i1],
                            rrep[:, i0:i1], MUL)
                        nc.vector.tensor_tensor(o4[:, i0:i1], bps[:], cr[:], MUL)
                        nc.gpsimd.tensor_tensor(
                            o3[:, i0:i1], o2[:, i0:i1],
                            cf[:, hc * LC + i0:hc * LC + i1], MUL)
                    # ---- stores ----
                    r0 = hc * 128
                    nc.sync.dma_start(out[b, r0:r0 + 128, :],
                                      cf[:, hc * LC:(hc + 1) * LC])
                    nc.sync.dma_start(out[b, H + r0:H + r0 + 128, :], o2[:])
                    nc.sync.dma_start(out[b, 2 * H + r0:2 * H + r0 + 128, :], o3[:])
                    nc.sync.dma_start(out[b, 3 * H + r0:3 * H + r0 + 128, :], o4[:])

    nc.compile()
    return nc


def _prep(C, Q, cmask, qmask, line_project):
    import ml_dtypes
    w1, w2, w3 = np.split(line_project.astype(np.float64), 3)
    r = np.einsum('bhi,h->bi', C.astype(np.float64), w1).astype(np.float32)
    c_ = np.einsum('bhj,h->bj', Q.astype(np.float64), w2).astype(np.float32)
    rm = (r - NEG * cmask).reshape(B, IC, 128).transpose(0, 2, 1).copy()
    cb = (c_ - NEG * qmask).reshape(B, JC, 128).transpose(0, 2, 1).copy()
    q3 = (Q * w3.astype(np.float32)[None, :, None]).astype(np.float32)
    qt = np.ascontiguousarray(Q.transpose(0, 2, 1))
    ct = np.empty((B, LC, HA), dtype=ml_dtypes.bfloat16)
    ct[:, :, :H] = C.transpose(0, 2, 1).astype(ml_dtypes.bfloat16)
    ct[:, :, H] = np.float32(1.0)
    return rm, cb, q3, qt, ct


def kernel(C, Q, cmask, qmask, line_project):
    from concourse.bass_utils import run_bass_kernel_spmd

    C = np.asarray(C, dtype=np.float32)
    Q = np.asarray(Q, dtype=np.float32)
    cmask = np.asarray(cmask, dtype=np.float32)
    qmask = np.asarray(qmask, dtype=np.float32)
    line_project = np.asarray(line_project, dtype=np.float32)

    rm, cb, q3, qt, ct = _prep(C, Q, cmask, qmask, line_project)

    if "nc" not in _CACHE:
        _CACHE["nc"] = _build()
    nc = _CACHE["nc"]

    in_maps = []
    for core in range(NCORES):
        s = slice(core * NB, (core + 1) * NB)
        in_maps.append({
            "c32": np.ascontiguousarray(C[s]),
            "ct": np.ascontiguousarray(ct[s]),
            "q3": np.ascontiguousarray(q3[s]),
            "qt": np.ascontiguousarray(qt[s]),
            "rm": np.ascontiguousarray(rm[s]),
            "cb": np.ascontiguousarray(cb[s]),
        })
    res = run_bass_kernel_spmd(nc, in_maps, core_ids=list(range(NCORES)))
    return np.concatenate([res.results[c]["out"] for c in range(NCORES)], axis=0)
